# revision 1
# baseline (speedup 1.0000x reference)
"""GAT head kernel for Trainium2, 8 NeuronCores (SPMD via bass).

Reference computation (B=4, N=4096, D=256):
    feats  = data @ W1.T                          [B,N,D]
    f1     = feats @ W2 + b2                      [B,N]
    logits = f1[:,:,None] + f1[:,None,:]          [B,N,N]
    coefs  = softmax(leaky_relu(logits) + bias1, axis=-1)
    out    = coefs @ feats + bias2 + data

Sharding: core c = 2*b + h owns batch b, row half h (2048 rows), needs all
N feats of its batch. Everything on-chip is laid out so the big N x N work
happens in "transposed" [j(partition), i(free)] tiles:

    E[j, i] = exp(leaky_relu(f1[i] + f1[j]) + bias1[i, j])      (unnormalized)

which is exactly the lhsT the PE wants for vals[i,:] = sum_j E[j,i]*feats[j,:].
A ones-column appended to the rhs makes the PE also produce the softmax
denominator D[i] on the same partition as vals[i,:].

leaky_relu under exp is handled exactly by monotonicity:
    exp(LR(x)) = max(exp(x), exp(0.01 x))
and exp(0.01 x) is either computed exactly (ACT pass) or approximated by
1 + 0.01 x (|0.01 x| < 0.1, rel err < 4e-3 only on down-weighted entries).

Host-side prep (layout only + exp(bias1)): transposes, tiling of small
vectors, exp of the additive bias so the on-device bias add becomes a
multiply that fuses into the bf16 pipeline.
"""

import sys

sys.path.insert(0, "/opt/trn_rl_repo")

import numpy as np
import ml_dtypes

import concourse.bass as bass
import concourse.mybir as mybir
from concourse.tile import TileContext
from concourse.bass_utils import run_bass_kernel_spmd

# ---------------------------------------------------------------- config
B, N, D = 4, 4096, 256
NCORES = 8
R = N * B // NCORES          # rows per core = 2048
NB = N // 128                # j blocks = 32
IC = 512                     # i-chunk width
NIC = R // IC                # i chunks per core = 4
HB = R // 128                # 16: i-blocks of 128 per core

F32 = mybir.dt.float32
BF16 = mybir.dt.bfloat16

EXACT_EB = False             # True: extra ACT pass for exp(0.01x); False: 1+0.01x approx
E_DT = BF16                  # dtype of E tiles / matmul operands
E_POOL_BUFS = 2
# engine-balancing knobs, applied per tile index k = ic*NB+jb (see build_nc):
# t = 1+0.01x on ACT (so the max is a 2x-mode TT on DVE) for k%DEN < ACT_T_NUM
ACT_T_NUM, ACT_T_DEN = 0, 4
# E = m*expb1 multiply on gpsimd for k%DEN < POOL_MULT_NUM
POOL_MULT_NUM, POOL_MULT_DEN = 0, 4
# max(ea, t) on gpsimd (only for act_t tiles) for k%DEN < POOL_MAX_NUM
POOL_MAX_NUM, POOL_MAX_DEN = 0, 4
# STT (m) on gpsimd (only for stt tiles) for k%DEN < POOL_STT_NUM
POOL_STT_NUM, POOL_STT_DEN = 0, 8
# fb cast copies on ACT (True) or DVE (False)
FB_ACT = False

_nc_cache = {}


def _legalize_waits(nc, max_inst_waits=1, max_ev_waits=2):
    """This walrus accepts <=1 sync wait on normal instructions and <=2 on
    EventSemaphore. Hoist extra waits into EVSEMs placed right before the
    over-subscribed instruction on the same engine (same queue => ordered)."""
    counter = 0
    for fn in nc.m.functions:
        for bb in fn.blocks:
            out = []
            changed = False
            for ins in bb.instructions:
                si = ins.sync_info
                waits = list(si.on_wait) if si and si.on_wait else []
                limit = (
                    max_ev_waits
                    if isinstance(ins, mybir.InstEventSemaphore)
                    else max_inst_waits
                )
                if len(waits) > limit:
                    extra, keep = waits[:-limit], waits[-limit:]
                    while extra:
                        chunk, extra = extra[:max_ev_waits], extra[max_ev_waits:]
                        counter += 1
                        ev = mybir.InstEventSemaphore(
                            name=f"waitsplit_{counter}", engine=ins.engine
                        )
                        ev.sync_info = mybir.SyncInfo(on_wait=chunk, on_update=[])
                        out.append(ev)
                        changed = True
                    ins.sync_info = mybir.SyncInfo(
                        on_wait=keep,
                        on_update=list(si.on_update) if si.on_update else [],
                    )
                out.append(ins)
            if changed:
                bb.instructions = out
    return nc


def build_nc():
    key = (EXACT_EB, E_DT, E_POOL_BUFS, IC, ACT_T_NUM, ACT_T_DEN,
           POOL_MULT_NUM, POOL_MULT_DEN, POOL_MAX_NUM, POOL_MAX_DEN,
           POOL_STT_NUM, POOL_STT_DEN, FB_ACT)
    if key in _nc_cache:
        return _nc_cache[key]

    nc = bass.Bass()
    AF = mybir.ActivationFunctionType
    OP = mybir.AluOpType

    dataT_d = nc.dram_tensor("dataT", [D, N], F32, kind="ExternalInput")
    datan_d = nc.dram_tensor("datan", [R, D], F32, kind="ExternalInput")
    w1t_d = nc.dram_tensor("w1t", [D, D + 1], F32, kind="ExternalInput")
    b2rep_d = nc.dram_tensor("b2rep", [128, 1], F32, kind="ExternalInput")
    bias2bc_d = nc.dram_tensor("bias2bc", [128, D], F32, kind="ExternalInput")
    expb1t_d = nc.dram_tensor("expb1t", [N, R], E_DT, kind="ExternalInput")
    ident_d = nc.dram_tensor("ident", [128, 128], F32, kind="ExternalInput")
    out_d = nc.dram_tensor("out", [R, D], F32, kind="ExternalOutput")

    with TileContext(nc) as tc:
        with (
            tc.tile_pool(name="persist", bufs=1) as pp,
            tc.tile_pool(name="epool", bufs=E_POOL_BUFS) as ep,
            tc.tile_pool(name="stream", bufs=6) as sp,
            tc.tile_pool(name="psum", bufs=3, space="PSUM") as psp,
            tc.tile_pool(name="psfeat", bufs=2, space="PSUM") as psf,
        ):
            # ---------------- phase 0: feats, f1, broadcast rows ----------
            # dataT first, in halves, so the first HB feats blocks (which the
            # f1 broadcast chain needs) start as early as possible
            dT0 = pp.tile([128, N], F32, tag="dT0")
            dT1 = pp.tile([128, N], F32, tag="dT1")
            HALF = HB * 128
            Q = HALF // 2
            nc.sync.dma_start(dT0[:, 0:Q], dataT_d[0:128, 0:Q])
            nc.sync.dma_start(dT1[:, 0:Q], dataT_d[128:256, 0:Q])

            # w1t is host-augmented to [256, 257]: cols 0..255 = W1[o, i],
            # col 256 = w_eff = W1.T @ W2, so the feats matmul also yields
            # raw f1 in psum col 256 -- no DVE reduction pass needed.
            # bias2 / b2 are added downstream (bias2bc into fb, 2*b2 into the
            # per-partition bias columns).
            w1t_lo = pp.tile([128, D + 1], F32, tag="w1lo")
            w1t_hi = pp.tile([128, D + 1], F32, tag="w1hi")
            nc.sync.dma_start(w1t_lo[:], w1t_d[0:128, :])
            nc.sync.dma_start(w1t_hi[:], w1t_d[128:256, :])
            b2rep = pp.tile([128, 1], F32, tag="b2rep")
            nc.sync.dma_start(b2rep[:], b2rep_d[:])
            bias2bc = pp.tile([128, D], F32, tag="bias2bc")
            nc.sync.dma_start(bias2bc[:], bias2bc_d[:])
            b2s = pp.tile([128, 1], F32, tag="b2s")
            nc.vector.tensor_scalar_mul(b2s[:], b2rep[:], 0.01)
            nc.sync.dma_start(dT0[:, Q:HALF], dataT_d[0:128, Q:HALF])
            nc.sync.dma_start(dT1[:, Q:HALF], dataT_d[128:256, Q:HALF])
            nc.sync.dma_start(dT0[:, HALF:N], dataT_d[0:128, HALF:N])
            nc.sync.dma_start(dT1[:, HALF:N], dataT_d[128:256, HALF:N])
            ident = pp.tile([128, 128], F32, tag="ident")
            nc.sync.dma_start(ident[:], ident_d[:])
            ones128 = pp.tile([128, 128], F32, tag="ones128")
            nc.vector.memset(ones128[:], 1.0)

            # feats (bf16, +bias2 folded) with ones column at [:, :, 256]
            fb = pp.tile([128, NB, D + 1], E_DT, tag="fb")
            nc.vector.memset(fb[:, :, D : D + 1], 1.0)
            f1 = pp.tile([128, NB], F32, tag="f1")       # raw f1 (no b2)
            f1b2 = pp.tile([128, NB], F32, tag="f1b2")       # f1 + 2*b2
            f1c001a = pp.tile([128, NB], F32, tag="f1c001a") # 0.01*f1 + 0.02*b2
            f1bc = pp.tile([128, R], F32, tag="f1bc")
            f1bc001p1 = pp.tile([128, R], E_DT, tag="f1bc001p1")

            for jb in range(NB):
                jsl = slice(jb * 128, (jb + 1) * 128)
                ps = psf.tile([128, D + 1], F32, tag="featps")
                nc.tensor.matmul(ps[:], dT0[:, jsl], w1t_lo[:], start=True, stop=False)
                nc.tensor.matmul(ps[:], dT1[:, jsl], w1t_hi[:], start=False, stop=True)
                # feats_bf = feats + bias2, cast to E_DT
                nc.vector.tensor_tensor(fb[:, jb, 0:D], ps[:, 0:D], bias2bc[:], OP.add)
                nc.vector.tensor_copy(f1[:, jb : jb + 1], ps[:, D : D + 1])
                if jb < HB:
                    # own-row broadcast chain, interleaved per block so it
                    # does not queue behind all the feats matmuls: replicate
                    # raw f1 col jb along free, PE-transpose -> f1bc block.
                    colt = sp.tile([128, 128], F32, tag="colt", bufs=3)
                    nc.vector.tensor_scalar_mul(
                        colt[:], ones128[:], f1[:, jb : jb + 1]
                    )
                    psb = psf.tile([128, 128], F32, tag="psb")
                    nc.tensor.transpose(psb[:], colt[:], ident[:])
                    nc.vector.tensor_copy(
                        f1bc[:, jb * 128 : (jb + 1) * 128], psb[:]
                    )
                    if not EXACT_EB:
                        nc.scalar.activation(
                            f1bc001p1[:, jb * 128 : (jb + 1) * 128], psb[:],
                            AF.Identity, bias=1.0, scale=0.01,
                        )
                if jb == HB - 1 or jb == NB - 1:
                    # bias-column halves, available as soon as their f1 cols are
                    hsl = slice(0, HB) if jb == HB - 1 else slice(HB, NB)
                    nc.vector.tensor_scalar(
                        f1b2[:, hsl], f1[:, hsl], b2rep[:, 0:1], None, OP.add
                    )
                    nc.vector.tensor_scalar(
                        f1c001a[:, hsl], f1[:, hsl], 0.01, b2s[:, 0:1],
                        OP.mult, OP.add,
                    )

            # ---------------- phase 1: E tiles + matmul ----------
            # expb1t viewed [p, jb, i] for batched loads of 8 j-blocks
            expb1_r = expb1t_d.rearrange("(g q p) i -> g p q i", q=8, p=128)
            datan_r = datan_d.rearrange("(rb p) o -> p rb o", p=128)
            out_r = out_d.rearrange("(rb p) o -> p rb o", p=128)
            for ic in range(NIC):
                icsl = slice(ic * IC, (ic + 1) * IC)
                e = ep.tile([128, NB, IC], E_DT, tag="e")
                ebtg = [None] * 4
                for g in range(4):
                    ebtg[g] = sp.tile([128, 8, IC], E_DT, bufs=2,
                                      name=f"ebtg{g}", tag=f"expb1g{g % 2}")
                    nc.sync.dma_start(
                        ebtg[g][:], expb1_r[g, :, :, ic * IC : (ic + 1) * IC]
                    )
                dnb = sp.tile([128, 4, D], F32, tag="dnb", bufs=2)
                nc.sync.dma_start(dnb[:], datan_r[:, ic * 4 : (ic + 1) * 4, :])
                obuf = sp.tile([128, 4, D], F32, tag="obuf", bufs=2)
                for jb in range(NB):
                    ebt = ebtg[jb // 8][:, jb % 8, :]
                    ea = sp.tile([128, IC], E_DT, tag="ea")
                    nc.scalar.activation(
                        ea[:], f1bc[:, icsl], AF.Exp,
                        bias=f1b2[:, jb : jb + 1], scale=1.0,
                    )
                    m = sp.tile([128, IC], E_DT, tag="m")
                    k = ic * NB + jb
                    if EXACT_EB:
                        eb = sp.tile([128, IC], E_DT, tag="eb")
                        nc.scalar.activation(
                            eb[:], f1bc[:, icsl], AF.Exp,
                            bias=f1c001a[:, jb : jb + 1], scale=0.01,
                        )
                        nc.vector.tensor_tensor(m[:], ea[:], eb[:], OP.max)
                    elif (k % ACT_T_DEN) < ACT_T_NUM:
                        # t = 1+0.01x on ACT; max is then a plain TT on DVE
                        t = sp.tile([128, IC], E_DT, tag="tt")
                        nc.scalar.activation(
                            t[:], f1bc001p1[:, icsl], AF.Identity,
                            bias=f1c001a[:, jb : jb + 1], scale=1.0,
                        )
                        nc.vector.tensor_tensor(m[:], ea[:], t[:], OP.max)
                    else:
                        # t via TS (2x bf16), max via TT (2x bf16): HW-measured
                        # much faster than one 1x STT
                        t = sp.tile([128, IC], E_DT, tag="tt")
                        nc.vector.tensor_scalar(
                            t[:], f1bc001p1[:, icsl], f1c001a[:, jb : jb + 1],
                            None, OP.add,
                        )
                        nc.vector.tensor_tensor(m[:], ea[:], t[:], OP.max)
                    mult_eng = (
                        nc.gpsimd
                        if (k % POOL_MULT_DEN) < POOL_MULT_NUM
                        else nc.vector
                    )
                    mult_eng.tensor_tensor(e[:, jb, :], m[:], ebt, OP.mult)

                for i128 in range(IC // 128):
                    acc = psp.tile([128, D + 1], F32, tag="acc")
                    for jb in range(NB):
                        nc.tensor.matmul(
                            acc[:],
                            e[:, jb, i128 * 128 : (i128 + 1) * 128],
                            fb[:, jb, :],
                            start=(jb == 0),
                            stop=(jb == NB - 1),
                        )
                    rcp = sp.tile([128, 1], F32, tag="rcp")
                    nc.vector.reciprocal(rcp[:], acc[:, D : D + 1])
                    nc.vector.scalar_tensor_tensor(
                        obuf[:, i128, :], acc[:, 0:D], rcp[:, 0:1],
                        dnb[:, i128, :], OP.mult, OP.add,
                    )
                nc.sync.dma_start(out_r[:, ic * 4 : (ic + 1) * 4, :], obuf[:])

    _legalize_waits(nc)
    _nc_cache[key] = nc
    return nc


def make_in_maps(data, bias1, W1, W2, b2, bias2):
    """Host-side sharding / layout prep. Core c = 2*b + h."""
    data = np.asarray(data, dtype=np.float32)
    bias1 = np.asarray(bias1, dtype=np.float32)
    W1 = np.asarray(W1, dtype=np.float32)
    W2 = np.asarray(W2, dtype=np.float32)
    b2 = np.asarray(b2, dtype=np.float32)
    bias2 = np.asarray(bias2, dtype=np.float32)

    edt = ml_dtypes.bfloat16 if E_DT == BF16 else np.float32
    expb1t = np.exp(bias1).T  # [j, i]
    # augmented weights: see build_nc comment
    w1t = np.zeros((D, D + 1), dtype=np.float32)
    w1t[:, 0:D] = W1.T
    w1t[:, D] = (W1.astype(np.float64).T @ W2.astype(np.float64)).astype(np.float32)
    b2rep = np.full((128, 1), 2.0 * b2[0], dtype=np.float32)
    bias2bc = np.tile(bias2[None, :], (128, 1))
    ident = np.eye(128, dtype=np.float32)

    in_maps = []
    for c in range(NCORES):
        b, h = divmod(c, 2)
        rows = slice(h * R, (h + 1) * R)
        # dataT with own rows rolled to the front so that the kernel's
        # f1[:, 0:16] always corresponds to this core's own rows.
        dT = data[b].T  # [D, N]
        if h == 1:
            dT = np.concatenate([dT[:, R:], dT[:, :R]], axis=1)
            eb = np.concatenate([expb1t[R:, rows], expb1t[:R, rows]], axis=0)
        else:
            eb = expb1t[:, rows]
        in_maps.append(
            {
                "dataT": np.ascontiguousarray(dT),
                "datan": np.ascontiguousarray(data[b, rows]),
                "w1t": w1t,
                "b2rep": b2rep,
                "bias2bc": bias2bc,
                "expb1t": np.ascontiguousarray(eb.astype(edt)),
                "ident": ident,
            }
        )
    return in_maps


def assemble(results):
    out = np.empty((B, N, D), dtype=np.float32)
    for c in range(NCORES):
        b, h = divmod(c, 2)
        out[b, h * R : (h + 1) * R, :] = results[c]["out"]
    return out


def kernel(data, bias1, W1, W2, b2, bias2):
    nc = build_nc()
    in_maps = make_in_maps(data, bias1, W1, W2, b2, bias2)
    res = run_bass_kernel_spmd(nc, in_maps, core_ids=list(range(NCORES)))
    return assemble(res.results)

